# revision 18
# baseline (speedup 1.0000x reference)
"""MoE (MoVaE) layer on 8 Trainium2 NeuronCores — expert-parallel Bass kernel.

Reference computation (per token, top-2 of 12 experts):
  scores = x @ router_w.T ; rw = softmax(scores)
  top2 -> renormalized weights; experts 0-7 are MLP (relu^2 MLP),
  experts 8-11 are vocab-embedding (VE) table lookups.

Sharding (hardcoded): core i owns MLP expert i and the vocab half i%2 of
VE table i//2. Every core replicates the router over all 4096 tokens; as
each 128-token tile is routed it is immediately dispatched: a running
exclusive-cumsum (triangular matmul + scalar base carried across tiles)
assigns compact slots, and an indirect-DMA scatter writes (token, weight)
rows into compact meta tables. The expert phases then gather only the
routed tokens, run the expert MLP (float32r matmuls) / VE lookup, scale
by the routing weight, and return compact rows + token indices. The host
scatters the compact per-core rows back into the full (B,T,C) output
(the "unshard" combine) and takes routing_weights from core 0.
"""
import numpy as np

import concourse.bass as bass
import concourse.tile as tile
from concourse import bacc, mybir, bass_utils
from concourse.masks import make_upper_triangular, make_identity

F32 = mybir.dt.float32
I32 = mybir.dt.int32

B, T, C = 2, 2048, 1024
N = B * T                 # 4096 tokens
H = 2048
E_MLP, E_VE, E_TOT = 8, 4, 12
V, VH = 16384, 8192       # vocab, vocab half per core
NT = N // 128             # 32 token tiles
CAPM = 896                # MLP dispatch capacity (max observed count 738)
CAPV = 512                # VE dispatch capacity (max observed count 362)
BIG = 60000.0             # slot offset for unselected tokens (-> OOB skip)
N_CORES = 8

# dtype for the heavy MLP matmul operands: float32r streams 4x faster than
# float32 on the PE (1 cycle/row at N>=256) at ~1e-4 relative error.
MM_DT = mybir.dt.float32r


def build_program():
    nc = bacc.Bacc("TRN2", target_bir_lowering=False, debug=False,
                   num_devices=N_CORES)
    di = lambda name, shape: nc.dram_tensor(name, shape, F32, kind="ExternalInput").ap()
    do = lambda name, shape: nc.dram_tensor(name, shape, F32, kind="ExternalOutput").ap()

    x = di("x", [N, C])
    xT = di("xT", [C, N])
    rwT = di("rwT", [C, E_TOT])
    fcT = di("fcT", [C, H])
    pjT = di("pjT", [H, C])
    veh = di("veh", [VH, C])
    tok2d = nc.dram_tensor("tok2d", [128, NT], I32, kind="ExternalInput").ap()
    oh_mlp = di("oh_mlp", [128, E_TOT])
    oh_ve = di("oh_ve", [128, E_TOT])
    velo = di("velo", [128, 1])

    rw_out = do("rw_out", [N, E_TOT])
    mlp_meta = do("mlp_meta", [CAPM, 2])
    mlp_rows = do("mlp_rows", [CAPM, C])
    ve_meta = do("ve_meta", [CAPV, 4])
    ve_rows = do("ve_rows", [CAPV, C])

    from contextlib import ExitStack
    with tile.TileContext(nc) as tc:
        with ExitStack() as ctx:
            _body(tc, nc, ctx,
                  x=x, xT=xT, rwT=rwT, fcT=fcT, pjT=pjT, veh=veh,
                  tok2d=tok2d, oh_mlp=oh_mlp, oh_ve=oh_ve, velo=velo,
                  rw_out=rw_out, mlp_meta=mlp_meta, mlp_rows=mlp_rows,
                  ve_meta=ve_meta, ve_rows=ve_rows)
    nc.compile()
    return nc


def _body(tc, nc, ctx, *, x, xT, rwT, fcT, pjT, veh, tok2d, oh_mlp, oh_ve,
          velo, rw_out, mlp_meta, mlp_rows, ve_meta, ve_rows):
    const = ctx.enter_context(tc.tile_pool(name="const", bufs=1))
    wpool = ctx.enter_context(tc.tile_pool(name="weights", bufs=1))
    xtp = ctx.enter_context(tc.tile_pool(name="xt", bufs=3))
    small = ctx.enter_context(tc.tile_pool(name="small", bufs=4))
    gsel = ctx.enter_context(tc.tile_pool(name="gsel", bufs=2))
    hpool = ctx.enter_context(tc.tile_pool(name="hid", bufs=1))
    ypool = ctx.enter_context(tc.tile_pool(name="y", bufs=2))
    stg = ctx.enter_context(tc.tile_pool(name="stg", bufs=2))
    psum = ctx.enter_context(tc.tile_pool(name="psum", bufs=2, space="PSUM"))

    AF = mybir.ActivationFunctionType
    OP = mybir.AluOpType

    # ---- constants ----
    s128 = const.tile([128, 128], F32)          # strictly upper triangular
    make_upper_triangular(nc, s128[:], val=1.0, diag=False)
    ones = const.tile([128, 128], F32)
    nc.vector.memset(ones[:], 1.0)
    ident = const.tile([128, 128], F32)
    make_identity(nc, ident[:])
    oh_m_sb = const.tile([128, E_TOT], F32)
    nc.sync.dma_start(oh_m_sb[:], oh_mlp[:])
    oh_v_sb = const.tile([128, E_TOT], F32)
    nc.sync.dma_start(oh_v_sb[:], oh_ve[:])
    velo_sb = const.tile([128, 1], F32)
    nc.sync.dma_start(velo_sb[:], velo[:])
    tok_sb = const.tile([128, NT], I32)
    nc.sync.dma_start(tok_sb[:], tok2d[:])
    tok_f = const.tile([128, NT], F32)
    nc.vector.tensor_copy(tok_f[:], tok_sb[:])
    iota_i = const.tile([128, NT], I32)
    nc.gpsimd.iota(iota_i[:], pattern=[[128, NT]], base=0, channel_multiplier=1)
    iota_f = const.tile([128, NT], F32)
    nc.vector.tensor_copy(iota_f[:], iota_i[:])

    # ---- resident weights (scalar-engine DMA queue: keeps the sync queue
    # free for the router's x-tile streams) ----
    w_r = wpool.tile([128, 8 * E_TOT], F32)          # router: 8 c-blocks x 12
    rwT_r = rwT.rearrange("(b p) e -> p b e", p=128)  # [128, 8, 12]
    nc.sync.dma_start(w_r[:].rearrange("p (b e) -> p b e", b=8), rwT_r)
    w_fc = wpool.tile([128, 8 * H], MM_DT)           # fc: 8 c-blocks x 2048
    for b in range(8):
        for h2 in range(2):
            stage = stg.tile([128, C], F32, tag="wstage")
            nc.scalar.dma_start(stage[:], fcT[b * 128:(b + 1) * 128, h2 * C:(h2 + 1) * C])
            nc.vector.tensor_copy(w_fc[:, b * H + h2 * C: b * H + (h2 + 1) * C], stage[:])
    w_pj = wpool.tile([128, 16 * C], MM_DT)          # proj: 16 h-blocks x 1024
    for j in range(16):
        stage = stg.tile([128, C], F32, tag="wstage")
        nc.scalar.dma_start(stage[:], pjT[j * 128:(j + 1) * 128, :])
        nc.vector.tensor_copy(w_pj[:, j * C:(j + 1) * C], stage[:])

    # ---- zero fill of meta outputs (host filters on weight != 0) ----
    zfill = const.tile([128, 16], F32)
    nc.vector.memset(zfill[:], 0.0)
    fill_insts = [
        nc.sync.dma_start(mlp_meta.rearrange("(p a) c -> p (a c)", p=128),
                          zfill[:, :CAPM * 2 // 128]),
        nc.sync.dma_start(ve_meta.rearrange("(p a) c -> p (a c)", p=128),
                          zfill[:, :CAPV * 4 // 128]),
    ]

    xT_view = xT.rearrange("(b p) n -> p b n", p=128)  # [128, 8, 4096]

    # running dispatch base (col 0 = MLP slot base, col 1 = VE), offset by BIG
    base2 = const.tile([1, 2], F32)
    nc.vector.memset(base2[:], BIG)
    mlp_scatters = []
    ve_scatters = []

    # ===== Phase B: router + top-2 + fused running-prefix dispatch =====
    for t in range(NT):
        xt = xtp.tile([128, 8, 128], F32)
        nc.sync.dma_start(xt[:], xT_view[:, :, t * 128:(t + 1) * 128])
        scores_ps = psum.tile([128, E_TOT], F32, tag="ps_small")
        for b in range(8):
            nc.tensor.matmul(scores_ps[:], lhsT=xt[:, b, :],
                             rhs=w_r[:, b * E_TOT:(b + 1) * E_TOT],
                             start=(b == 0), stop=(b == 7))
        # softmax over the 12 experts
        negmax = small.tile([128, 1], F32, tag="row1")
        nc.vector.tensor_reduce(negmax[:], scores_ps[:], axis=mybir.AxisListType.X,
                                op=OP.max, negate=True)
        rwt = small.tile([128, E_TOT], F32, tag="rwt")
        sumexp = small.tile([128, 1], F32, tag="row2")
        nc.scalar.activation(rwt[:], scores_ps[:], AF.Exp,
                             bias=negmax[:], scale=1.0, accum_out=sumexp[:])
        recip = small.tile([128, 1], F32, tag="row3")
        nc.vector.reciprocal(recip[:], sumexp[:])
        nc.vector.tensor_scalar_mul(rwt[:], rwt[:], recip[:])
        nc.sync.dma_start(rw_out[t * 128:(t + 1) * 128, :], rwt[:])
        # top-2: m8[:,0:2] are the two largest routing weights
        m8 = small.tile([128, 8], F32, tag="m8")
        nc.vector.max(out=m8[:], in_=rwt[:])
        den = small.tile([128, 1], F32, tag="row4")
        nc.vector.tensor_add(den[:], m8[:, 0:1], m8[:, 1:2])
        nc.vector.tensor_scalar_add(den[:], den[:], 1e-10)
        rden = small.tile([128, 1], F32, tag="row5")
        nc.vector.reciprocal(rden[:], den[:])
        # my two experts' raw routing weights -> [128, 2]
        tmp24 = small.tile([128, 2, E_TOT], F32, tag="tmp24")
        nc.vector.tensor_mul(tmp24[:, 0, :], rwt[:], oh_m_sb[:])
        nc.vector.tensor_mul(tmp24[:, 1, :], rwt[:], oh_v_sb[:])
        my2 = small.tile([128, 2], F32, tag="my2")
        nc.vector.tensor_reduce(my2[:], tmp24[:], axis=mybir.AxisListType.X, op=OP.add)
        # selected iff my weight >= 2nd max (col 0: MLP, col 1: VE)
        f2 = small.tile([128, 2], F32, tag="f2")
        nc.vector.tensor_tensor(f2[:], my2[:], m8[:, 1:2].to_broadcast([128, 2]),
                                op=OP.is_ge)
        # VE also requires token id in my vocab half
        relf = small.tile([128, 1], F32, tag="row6")
        nc.vector.tensor_sub(relf[:], tok_f[:, t:t + 1], velo_sb[:])
        ge = small.tile([128, 1], F32, tag="row7")
        nc.vector.tensor_scalar(ge[:], relf[:], -0.5, None, op0=OP.is_gt)
        le = small.tile([128, 1], F32, tag="row8")
        nc.vector.tensor_scalar(le[:], relf[:], VH - 0.5, None, op0=OP.is_lt)
        nc.vector.tensor_mul(ge[:], ge[:], le[:])
        nc.vector.tensor_mul(f2[:, 1:2], f2[:, 1:2], ge[:])
        # renormalized weights
        w2 = small.tile([128, 2], F32, tag="w2")
        nc.vector.tensor_scalar_mul(w2[:], my2[:], rden[:])
        # slots = tile-local exclusive cumsum + running base (+BIG when !flag)
        slot_ps = psum.tile([128, 2], F32, tag="ps_slot")
        nc.tensor.matmul(slot_ps[:], lhsT=s128[:], rhs=f2[:], start=True, stop=False)
        nc.tensor.matmul(slot_ps[:], lhsT=ones[0:1, :], rhs=base2[:],
                         start=False, stop=True, skip_group_check=True)
        slotf = small.tile([128, 2], F32, tag="slotf")
        nc.vector.scalar_tensor_tensor(slotf[:], f2[:], -BIG, slot_ps[:],
                                       op0=OP.mult, op1=OP.add)
        islm = small.tile([128, 1], I32, tag="islm")
        nc.vector.tensor_copy(islm[:], slotf[:, 0:1])
        islv = small.tile([128, 1], I32, tag="islv")
        nc.vector.tensor_copy(islv[:], slotf[:, 1:2])
        # carry the base forward: base2 += per-tile counts
        colsum_ps = psum.tile([1, 2], F32, tag="ps_cs")
        nc.tensor.matmul(colsum_ps[:], lhsT=ones[:, 0:1], rhs=f2[:],
                         start=True, stop=True)
        nc.vector.tensor_add(base2[:], base2[:], colsum_ps[:])
        # pack + scatter (MLP: [token, w]; VE: [token, relrow, w, -])
        pkm = small.tile([128, 2], F32, tag="pkm")
        nc.vector.tensor_copy(pkm[:, 0:1], iota_f[:, t:t + 1])
        nc.vector.tensor_copy(pkm[:, 1:2], w2[:, 0:1])
        si = nc.gpsimd.indirect_dma_start(
            out=mlp_meta[:],
            out_offset=bass.IndirectOffsetOnAxis(ap=islm[:, 0:1], axis=0),
            in_=pkm[:], in_offset=None, bounds_check=CAPM - 1, oob_is_err=False)
        tile.add_dep_helper(si.ins, fill_insts[0].ins, reason="scatter after fill")
        mlp_scatters.append(si)
        pkv = small.tile([128, 4], F32, tag="pkv")
        nc.vector.tensor_copy(pkv[:, 0:1], iota_f[:, t:t + 1])
        nc.vector.tensor_copy(pkv[:, 1:2], relf[:])
        nc.vector.tensor_copy(pkv[:, 2:3], w2[:, 1:2])
        si = nc.gpsimd.indirect_dma_start(
            out=ve_meta[:],
            out_offset=bass.IndirectOffsetOnAxis(ap=islv[:, 0:1], axis=0),
            in_=pkv[:], in_offset=None, bounds_check=CAPV - 1, oob_is_err=False)
        tile.add_dep_helper(si.ins, fill_insts[1].ins, reason="scatter after fill")
        ve_scatters.append(si)

    # ================= Phase D: MLP expert on gathered tokens =================
    for c in range(CAPM // 128):
        meta_sb = small.tile([128, 2], F32, tag="metam")
        mi = nc.sync.dma_start(meta_sb[:], mlp_meta[c * 128:(c + 1) * 128, :])
        for si in mlp_scatters:
            tile.add_dep_helper(mi.ins, si.ins, reason="meta read waits on scatter")
        idx_i = small.tile([128, 1], I32, tag="idxm")
        nc.vector.tensor_copy(idx_i[:], meta_sb[:, 0:1])
        x_sel = gsel.tile([128, C], F32, tag="xsel")
        nc.gpsimd.indirect_dma_start(
            out=x_sel[:], out_offset=None, in_=x[:],
            in_offset=bass.IndirectOffsetOnAxis(ap=idx_i[:, 0:1], axis=0),
            bounds_check=N - 1, oob_is_err=False)
        xTs = gsel.tile([128, C], MM_DT, tag="xTs")
        for b in range(8):
            tr_ps = psum.tile([128, 128], F32, tag="ps_tr")
            nc.tensor.transpose(tr_ps[:], x_sel[:, b * 128:(b + 1) * 128], ident[:])
            nc.vector.tensor_copy(xTs[:, b * 128:(b + 1) * 128], tr_ps[:])
        hid = hpool.tile([128, H], F32, tag="hid")
        for hb in range(4):
            mm_ps = psum.tile([128, 512], F32, tag="ps_mm")
            for cb in range(8):
                nc.tensor.matmul(mm_ps[:], lhsT=xTs[:, cb * 128:(cb + 1) * 128],
                                 rhs=w_fc[:, cb * H + hb * 512: cb * H + (hb + 1) * 512],
                                 start=(cb == 0), stop=(cb == 7))
            # relu^2: ACT relu (PSUM -> SBUF), then DVE square in SBUF
            rt = hpool.tile([128, 512], F32, tag="relu_t")
            nc.scalar.activation(rt[:], mm_ps[:], AF.Relu)
            nc.vector.tensor_mul(hid[:, hb * 512:(hb + 1) * 512], rt[:], rt[:])
        hTs = hpool.tile([128, H], MM_DT, tag="hTs")
        for j in range(16):
            tr_ps = psum.tile([128, 128], F32, tag="ps_tr")
            nc.tensor.transpose(tr_ps[:], hid[:, j * 128:(j + 1) * 128], ident[:])
            nc.vector.tensor_copy(hTs[:, j * 128:(j + 1) * 128], tr_ps[:])
        y = ypool.tile([128, C], F32, tag="y")
        for cb2 in range(2):
            mm_ps = psum.tile([128, 512], F32, tag="ps_mm")
            for j in range(16):
                nc.tensor.matmul(mm_ps[:], lhsT=hTs[:, j * 128:(j + 1) * 128],
                                 rhs=w_pj[:, j * C + cb2 * 512: j * C + (cb2 + 1) * 512],
                                 start=(j == 0), stop=(j == 15))
            nc.vector.tensor_scalar_mul(y[:, cb2 * 512:(cb2 + 1) * 512],
                                        mm_ps[:], meta_sb[:, 1:2])
        nc.sync.dma_start(mlp_rows[c * 128:(c + 1) * 128, :], y[:])

    # ================= Phase E: VE lookups =================
    for c in range(CAPV // 128):
        meta4 = small.tile([128, 4], F32, tag="metav")
        mi = nc.sync.dma_start(meta4[:], ve_meta[c * 128:(c + 1) * 128, :])
        for si in ve_scatters:
            tile.add_dep_helper(mi.ins, si.ins, reason="meta read waits on scatter")
        rel_i = small.tile([128, 1], I32, tag="idxv")
        nc.vector.tensor_copy(rel_i[:], meta4[:, 1:2])
        ve_sel = gsel.tile([128, C], F32, tag="xsel")
        nc.gpsimd.indirect_dma_start(
            out=ve_sel[:], out_offset=None, in_=veh[:],
            in_offset=bass.IndirectOffsetOnAxis(ap=rel_i[:, 0:1], axis=0),
            bounds_check=VH - 1, oob_is_err=False)
        yv = ypool.tile([128, C], F32, tag="y")
        nc.vector.tensor_scalar_mul(yv[:], ve_sel[:], meta4[:, 2:3])
        nc.sync.dma_start(ve_rows[c * 128:(c + 1) * 128, :], yv[:])


_NC_CACHE = None
_LAST_RESULTS = None  # BassKernelResults of the most recent kernel() call


def _get_nc():
    global _NC_CACHE
    if _NC_CACHE is None:
        _NC_CACHE = build_program()
    return _NC_CACHE


def kernel(**inputs):
    x = np.ascontiguousarray(np.asarray(inputs["x"], dtype=np.float32).reshape(N, C))
    tok = np.asarray(inputs["token_ids"]).reshape(N).astype(np.int32)
    router_w = np.asarray(inputs["router_w"], dtype=np.float32)
    fc_w = np.asarray(inputs["fc_w"], dtype=np.float32)
    proj_w = np.asarray(inputs["proj_w"], dtype=np.float32)
    ve_tables = np.asarray(inputs["ve_tables"], dtype=np.float32)

    nc = _get_nc()
    xT = np.ascontiguousarray(x.T)
    rwT = np.ascontiguousarray(router_w.T)
    tok2d = np.ascontiguousarray(tok.reshape(NT, 128).T).astype(np.int32)

    in_maps = []
    for i in range(N_CORES):
        vi, half = i // 2, i % 2
        oh_m = np.zeros((128, E_TOT), np.float32)
        oh_m[:, i] = 1.0
        oh_v = np.zeros((128, E_TOT), np.float32)
        oh_v[:, E_MLP + vi] = 1.0
        in_maps.append({
            "x": x,
            "xT": xT,
            "rwT": rwT,
            "fcT": np.ascontiguousarray(fc_w[i].T),
            "pjT": np.ascontiguousarray(proj_w[i].T),
            "veh": np.ascontiguousarray(ve_tables[vi, half * VH:(half + 1) * VH]),
            "tok2d": tok2d,
            "oh_mlp": oh_m,
            "oh_ve": oh_v,
            "velo": np.full((128, 1), half * VH, np.float32),
        })

    res = bass_utils.run_bass_kernel_spmd(nc, in_maps, core_ids=list(range(N_CORES)))
    global _LAST_RESULTS
    _LAST_RESULTS = res

    out = np.zeros((N, C), np.float32)
    for i, r in enumerate(res.results):
        for meta_k, rows_k, wcol in (("mlp_meta", "mlp_rows", 1), ("ve_meta", "ve_rows", 2)):
            meta = r[meta_k]
            rows = r[rows_k]
            w = meta[:, wcol]
            valid = w != 0.0
            cnt = int(valid.sum())
            if cnt >= meta.shape[0]:
                raise RuntimeError(f"core {i} {meta_k}: dispatch capacity saturated")
            idx = meta[valid, 0].astype(np.int64)
            if idx.size and (idx.min() < 0 or idx.max() >= N):
                raise RuntimeError(f"core {i} {meta_k}: bad token indices")
            out[idx] += rows[valid]

    routing_weights = res.results[0]["rw_out"].reshape(B, T, E_TOT)
    return out.reshape(B, T, C), routing_weights
